# revision 19
# baseline (speedup 1.0000x reference)
"""Causal self-attention kernel for Trainium2, sharded over 8 NeuronCores.

Problem: x:(2048,2,768) f32, 12 heads, head_dim 64.
Sharding: batch (2) x head-groups (4 groups of 3 heads) -> 8 cores.
Each core computes q/k/v projections for its (batch, 3 heads), causal
flash-style attention, and a partial c_proj contribution. The host sums the
4 partial outputs per batch (the "all-reduce") and adds bo.

Device-side layout notes:
  - Matmul operands are bf16 (1 cycle/row on PE; fp32 accumulate in PSUM).
  - Scores are computed TRANSPOSED: scoresT[t, s] so softmax's denominator
    comes from a ones-column appended to V (m=65 matmul) and the exp runs
    along the free axis; no PE transposes of the probability matrix needed.
  - Causal masking: diagonal 128x512 score tiles restrict the live column
    range (lo) and a gpsimd affine_select zeroes the triangular remainder.
  - x is DMA'd in (sblock, ktile) chunks across two queues so the first
    projection matmuls start as soon as chunk (0, sb0) lands; weights for
    q/k/v plus a concatenated [q_tail|k_tail] block arrive as one tile per
    ktile ("wall").
  - Query blocks are processed in order 0,1,3,2 with deferred c_proj blocks
    woven between attention tiles so the PE stays fed in the exp-paced
    stretches, and per-head divides start as soon as that head's last AV
    matmul retires.
"""

import os
import sys

sys.path.insert(0, "/opt/trn_rl_repo")

import numpy as np

import concourse.bass as bass  # noqa: F401  (import keeps bass registered)
import concourse.tile as tile
from concourse import bacc, bass_utils, library_config, mybir

F32 = mybir.dt.float32
BF16 = mybir.dt.bfloat16

S = 2048          # sequence length
B = 2             # batch
D = 768           # d_model
H = 12            # total heads
HD = 64           # head dim
NH = 3            # heads per core
DKL = NH * HD     # local q/k/v width = 192
KT = 6            # k-tiles over D (6 x 128)
SB = 512          # s-block width
QB = S // SB      # 4 q-blocks
TT = S // 128     # 16 t-tiles
WQK = 3 * DKL + 128  # wall width: q|k|v|qtail|ktail = 704
SCALE = 1.0 / np.sqrt(HD)

_PROGRAM_CACHE = {}
LAST_EXEC_NS = None


def _build_program():
    nc = bacc.Bacc("TRN2", target_bir_lowering=False, debug=False, num_devices=8)

    # x chunk-major: [sblock, ktile, 128, 512]
    xt_d = nc.dram_tensor("xt", [QB, KT, 128, SB], BF16, kind="ExternalInput").ap()
    # per-ktile weight wall: q(0:192) | k(192:384) | v(384:576) | qk_tail(576:704)
    wall_d = nc.dram_tensor("wall", [KT, 128, WQK], BF16, kind="ExternalInput").ap()
    wo_d = nc.dram_tensor("wo", [DKL, D], BF16, kind="ExternalInput").ap()
    ones_d = nc.dram_tensor("ones3", [128, NH, 1], BF16, kind="ExternalInput").ap()
    out_d = nc.dram_tensor("outT", [D, S], BF16, kind="ExternalOutput").ap()

    EXP = mybir.ActivationFunctionType.Exp
    GE = mybir.AluOpType.is_ge
    MUL = mybir.AluOpType.mult

    with tile.TileContext(nc) as tc:
        with (
            tc.tile_pool(name="xp", bufs=1) as xp,
            tc.tile_pool(name="wp", bufs=1) as wp,
            tc.tile_pool(name="qk", bufs=1) as qk,
            tc.tile_pool(name="vp", bufs=1) as vp,
            tc.tile_pool(name="ep", bufs=10) as ep,
            tc.tile_pool(name="ys", bufs=1) as ys,
            tc.tile_pool(name="dn", bufs=4) as dn,
            tc.tile_pool(name="op", bufs=4) as op,
            tc.tile_pool(name="psA", bufs=3, space="PSUM") as psA,
            tc.tile_pool(name="psB", bufs=3, space="PSUM") as psB,
            tc.tile_pool(name="psC", bufs=2, space="PSUM") as psC,
        ):
            nc.gpsimd.load_library(library_config.attn)

            # ---- Phase 0: input DMAs on two queues, first-needed first ----
            walls = []
            for k in range(KT):
                t = wp.tile([128, WQK], BF16, tag=f"wall{k}")
                walls.append(t)
            # one tile per (ktile, sblock) chunk so consumers only wait
            # for the chunks they actually read (DMA deps are tile-granular)
            xc = [
                [
                    xp.tile([128, SB], BF16, tag=f"x{k}_{n}", name=f"xc{k}_{n}")
                    for n in range(QB)
                ]
                for k in range(KT)
            ]

            # sync queue: weight walls, then x sblocks 2,3
            # gpsimd queue: x sblocks 0,1 (gate the prefix)
            for k in range(KT):
                nc.sync.dma_start(walls[k][:], wall_d[k])
            for n in (0, 1):
                for k in range(KT):
                    nc.gpsimd.dma_start(xc[k][n][:], xt_d[n, k])
            for n in (2, 3):
                for k in range(KT):
                    nc.sync.dma_start(xc[k][n][:], xt_d[n, k])
            # vector queue: c_proj weights (needed only mid-stream)
            wo1 = wp.tile([128, D], BF16, tag="wo1")
            nc.scalar.dma_start(wo1[:], wo_d[0:128])
            wo2 = wp.tile([64, D], BF16, tag="wo2")
            nc.scalar.dma_start(wo2[:], wo_d[128:DKL])

            qA = qk.tile([128, S], BF16, tag="qA")
            qB_ = qk.tile([64, S], BF16, tag="qB")
            kA = qk.tile([128, S], BF16, tag="kA")
            kB = qk.tile([64, S], BF16, tag="kB")
            yA = ys.tile([128, S], BF16, tag="yA")
            yB = ys.tile([64, S], BF16, tag="yB")

            vas = [None] * TT

            def qkv_groups(ncol):
                c0, c1 = ncol * SB, (ncol + 1) * SB

                def g_q(n=ncol):
                    ps = psC.tile([128, SB], F32, tag="mm", name=f"pq_{n}")
                    for k in range(KT):
                        nc.tensor.matmul(
                            ps[:], walls[k][:, 0:128], xc[k][n][:],
                            start=(k == 0), stop=(k == KT - 1),
                        )
                    nc.vector.tensor_copy(qA[:, c0:c1], ps[:])

                def g_k(n=ncol):
                    ps = psC.tile([128, SB], F32, tag="mm", name=f"pk_{n}")
                    for k in range(KT):
                        nc.tensor.matmul(
                            ps[:], walls[k][:, 192:320], xc[k][n][:],
                            start=(k == 0), stop=(k == KT - 1),
                        )
                    nc.vector.tensor_copy(kA[:, c0:c1], ps[:])

                def g_tail(n=ncol):
                    # one full-array matmul: rows 0:64 = q cols 128:192,
                    # rows 64:128 = k cols 128:192 (concatenated weights)
                    ps = psC.tile([128, SB], F32, tag="mm", name=f"pt_{n}")
                    for k in range(KT):
                        nc.tensor.matmul(
                            ps[:], walls[k][:, 576:704], xc[k][n][:],
                            start=(k == 0), stop=(k == KT - 1),
                        )
                    nc.vector.tensor_copy(qB_[:, c0:c1], ps[0:64, :])
                    nc.vector.tensor_copy(kB[:, c0:c1], ps[64:128, :])

                def mk_v(t):
                    def g_v():
                        ps = psC.tile([128, SB], F32, tag="mm", name=f"pv{t}")
                        for k in range(KT):
                            nc.tensor.matmul(
                                ps[:, 0:DKL],
                                xc[k][t // 4][:, (t % 4) * 128 : (t % 4 + 1) * 128],
                                walls[k][:, 384:576],
                                start=(k == 0), stop=(k == KT - 1),
                            )
                        va = vp.tile(
                            [128, NH * (HD + 1)], BF16, tag=f"v{t}", name=f"va{t}"
                        )
                        var = va[:].rearrange("p (h c) -> p h c", c=HD + 1)
                        nc.gpsimd.dma_start(var[:, :, HD : HD + 1], ones_d)
                        nc.vector.tensor_copy(
                            var[:, :, 0:HD],
                            ps[:, 0:DKL].rearrange("p (h d) -> p h d", d=HD),
                        )
                        vas[t] = va
                    return g_v

                return [g_q, g_k, g_tail] + [mk_v(t) for t in range(4 * ncol, 4 * ncol + 4)]

            def cproj_groups(qb):
                def mk(mc):
                    def g():
                        ps = psC.tile([128, SB], F32, tag="mm", name=f"cp_{qb}_{mc}")
                        nc.tensor.matmul(
                            ps[:],
                            wo1[:, mc * 128 : (mc + 1) * 128],
                            yA[:, qb * SB : (qb + 1) * SB],
                            start=True, stop=False,
                        )
                        nc.tensor.matmul(
                            ps[:],
                            wo2[:, mc * 128 : (mc + 1) * 128],
                            yB[:, qb * SB : (qb + 1) * SB],
                            start=False, stop=True,
                        )
                        st = op.tile([128, SB], BF16, tag="st", name=f"st_{qb}_{mc}")
                        nc.vector.tensor_copy(st[:], ps[:])
                        nc.sync.dma_start(
                            out_d[mc * 128 : (mc + 1) * 128, qb * SB : (qb + 1) * SB],
                            st[:],
                        )
                    return g
                return [mk(mc) for mc in range(D // 128)]

            def att_tile(qb, t, yps, nt):
                d = t * 128 - qb * SB
                lo, sw = (d, 128) if d >= 0 else (0, 0)
                sq = qA[:, qb * SB + lo : (qb + 1) * SB]
                sqB = qB_[:, qb * SB + lo : (qb + 1) * SB]
                sps = [
                    psA.tile([128, SB], F32, tag="sc", name=f"sp_{qb}_{t}_{h}")
                    for h in range(NH)
                ]
                nc.tensor.matmul(
                    sps[0][:, lo:SB],
                    kA[0:64, t * 128 : (t + 1) * 128],
                    sq[0:64, :], start=True, stop=True,
                )
                nc.tensor.matmul(
                    sps[1][:, lo:SB],
                    kA[64:128, t * 128 : (t + 1) * 128],
                    sq[64:128, :], start=True, stop=True,
                )
                nc.tensor.matmul(
                    sps[2][:, lo:SB],
                    kB[0:64, t * 128 : (t + 1) * 128],
                    sqB[0:64, :], start=True, stop=True,
                )
                for h in range(NH):
                    ex = ep.tile([128, SB], BF16, tag="exp", name=f"ex_{qb}_{t}_{h}")
                    nc.scalar.activation(
                        ex[:, lo:SB], sps[h][:, lo:SB], EXP, scale=float(SCALE)
                    )
                    if d >= 0:
                        nc.gpsimd.affine_select(
                            out=ex[:, lo : lo + sw],
                            in_=ex[:, lo : lo + sw],
                            compare_op=GE, fill=0.0,
                            base=0, channel_multiplier=-1,
                            pattern=[[1, sw]],
                        )
                    nc.tensor.matmul(
                        yps[h][:, lo:SB],
                        vas[t][:, h * (HD + 1) : (h + 1) * (HD + 1)],
                        ex[:, lo:SB],
                        start=(t == 0), stop=(t == nt - 1),
                    )

            def divide_h(qb, h, yps):
                dr = dn.tile([1, SB], F32, tag="dr", name=f"dr{qb}{h}")
                nc.vector.tensor_copy(dr[:], yps[h][HD : HD + 1, :])
                rc = dn.tile([1, SB], F32, tag="rc", name=f"rc{qb}{h}")
                nc.vector.reciprocal_approx_fast(rc[:], dr[:])
                bc = dn.tile([64, SB], F32, tag="bc", name=f"bc{qb}{h}")
                nc.gpsimd.partition_broadcast(bc[:], rc[:], channels=64)
                if h == 0:
                    dst = yA[0:64, qb * SB : (qb + 1) * SB]
                elif h == 1:
                    dst = yA[64:128, qb * SB : (qb + 1) * SB]
                else:
                    dst = yB[0:64, qb * SB : (qb + 1) * SB]
                nc.vector.tensor_tensor(dst, yps[h][0:HD, :], bc[:], MUL)

            from collections import deque

            # prefix: projections for sblocks 0 and 1 (paced by input DMA)
            for g in qkv_groups(0):
                g()
            for g in qkv_groups(1):
                g()

            # attention windows: (qb, background PE work woven between tiles).
            # Order 1,3,2,0 so the last window is the small, PE-bound all-diag
            # qb=0 block and its c_proj tail runs on a warm PE.
            windows = [
                (0, list(qkv_groups(2))),
                (1, list(qkv_groups(3))),
                (3, cproj_groups(0) + cproj_groups(1)),
                (2, cproj_groups(3)),
            ]
            for qb, bgl in windows:
                bg = deque(bgl)
                nt = 4 * qb + 4
                yps = [
                    psB.tile([HD + 1, SB], F32, tag="ya", name=f"yps_{qb}_{h}")
                    for h in range(NH)
                ]
                nbg = len(bg)
                emitted = 0
                for i in range(nt):
                    att_tile(qb, i, yps, nt)
                    if i == nt - 1:
                        for h in range(NH):
                            divide_h(qb, h, yps)
                    want = (i + 1) * nbg // nt
                    while emitted < want and bg:
                        bg.popleft()()
                        emitted += 1
            for g in cproj_groups(2):
                g()

    nc.compile()
    return nc


def kernel(x, Wq, bq, Wk, bk, Wv, bv, Wo, bo):
    global LAST_EXEC_NS
    x = np.asarray(x, dtype=np.float32)
    Wq = np.asarray(Wq, dtype=np.float32)
    Wk = np.asarray(Wk, dtype=np.float32)
    Wv = np.asarray(Wv, dtype=np.float32)
    Wo = np.asarray(Wo, dtype=np.float32)
    bq = np.asarray(bq, dtype=np.float32)
    bk = np.asarray(bk, dtype=np.float32)
    bv = np.asarray(bv, dtype=np.float32)
    bo = np.asarray(bo, dtype=np.float32)

    # The device program folds no biases; handle the (unused in this problem)
    # nonzero case on the host by a reference fallback.
    if np.any(bq) or np.any(bk) or np.any(bv):
        q = (x @ Wq + bq).reshape(S, B, H, HD)
        k = (x @ Wk + bk).reshape(S, B, H, HD)
        v = (x @ Wv + bv).reshape(S, B, H, HD)
        att = np.einsum("sbhd,tbhd->bhst", q, k) * SCALE
        causal = np.triu(np.ones((S, S), dtype=bool), k=1)
        att = np.where(causal[None, None], -np.inf, att)
        att = att - att.max(axis=-1, keepdims=True)
        att = np.exp(att)
        att = att / att.sum(axis=-1, keepdims=True)
        y = np.einsum("bhst,tbhd->sbhd", att, v).reshape(S, B, D)
        return (y @ Wo + bo).astype(np.float32)

    if "prog" not in _PROGRAM_CACHE:
        _PROGRAM_CACHE["prog"] = _build_program()
    nc = _PROGRAM_CACHE["prog"]

    import ml_dtypes

    bf = ml_dtypes.bfloat16
    in_maps = []
    # x chunk-major: [sblock, ktile, 128, 512]
    xT = [
        np.ascontiguousarray(
            x[:, b, :].T.astype(bf).reshape(KT, 128, QB, SB).transpose(2, 0, 1, 3)
        )
        for b in range(B)
    ]
    for c in range(8):
        b, g = c // 4, c % 4
        sl = slice(g * DKL, (g + 1) * DKL)
        wq_l = Wq[:, sl].astype(bf).reshape(KT, 128, DKL)
        wk_l = Wk[:, sl].astype(bf).reshape(KT, 128, DKL)
        wv_l = Wv[:, sl].astype(bf).reshape(KT, 128, DKL)
        wall = np.ascontiguousarray(
            np.concatenate(
                [wq_l, wk_l, wv_l, wq_l[:, :, 128:], wk_l[:, :, 128:]], axis=2
            )
        )
        in_maps.append({
            "xt": xT[b],
            "wall": wall,
            "wo": np.ascontiguousarray(Wo[sl, :]).astype(bf),
            "ones3": np.ones((128, NH, 1), dtype=bf),
        })

    trace = bool(int(os.environ.get("KERNEL_TRACE", "0")))
    res = bass_utils.run_bass_kernel_spmd(
        nc, in_maps, core_ids=list(range(8)), trace=trace
    )
    LAST_EXEC_NS = res.exec_time_ns

    out = np.zeros((S, B, D), dtype=np.float32)
    for c in range(8):
        b = c // 4
        out[:, b, :] += res.results[c]["outT"].astype(np.float32).T
    out += bo
    return out


# revision 21
# speedup vs baseline: 1.1896x; 1.1896x over previous
"""Causal self-attention kernel for Trainium2, sharded over 8 NeuronCores.

Problem: x:(2048,2,768) f32, 12 heads, head_dim 64.
Sharding: batch (2) x head-groups (4 groups of 3 heads) -> 8 cores.
Each core computes q/k/v projections for its (batch, 3 heads), causal
flash-style attention, and a partial c_proj contribution. The host sums the
4 partial outputs per batch (the "all-reduce") and adds bo.

Device-side layout notes:
  - Matmul operands are bf16 (1 cycle/row on PE; fp32 accumulate in PSUM).
  - Scores are computed TRANSPOSED: scoresT[t, s] so softmax's denominator
    comes from a ones-column appended to V (m=65 matmul) and the exp runs
    along the free axis; no PE transposes of the probability matrix needed.
  - Causal masking: diagonal 128x512 score tiles restrict the live column
    range (lo) and a gpsimd affine_select zeroes the triangular remainder.
  - x is DMA'd in (sblock, ktile) chunks across two queues so the first
    projection matmuls start as soon as chunk (0, sb0) lands; weights for
    q/k/v plus a concatenated [q_tail|k_tail] block arrive as one tile per
    ktile ("wall").
  - Query blocks are processed in order 0,1,3,2 with deferred c_proj blocks
    woven between attention tiles so the PE stays fed in the exp-paced
    stretches, and per-head divides start as soon as that head's last AV
    matmul retires.
"""

import os
import sys

sys.path.insert(0, "/opt/trn_rl_repo")

import numpy as np

import concourse.bass as bass  # noqa: F401  (import keeps bass registered)
import concourse.tile as tile
from concourse import bacc, bass_utils, library_config, mybir

F32 = mybir.dt.float32
BF16 = mybir.dt.bfloat16

S = 2048          # sequence length
B = 2             # batch
D = 768           # d_model
H = 12            # total heads
HD = 64           # head dim
NH = 3            # heads per core
DKL = NH * HD     # local q/k/v width = 192
KT = 6            # k-tiles over D (6 x 128)
SB = 512          # s-block width
QB = S // SB      # 4 q-blocks
TT = S // 128     # 16 t-tiles
WQK = 3 * DKL + 128  # wall width: q|k|v|qtail|ktail = 704
SCALE = 1.0 / np.sqrt(HD)

_PROGRAM_CACHE = {}
LAST_EXEC_NS = None


def _build_program():
    nc = bacc.Bacc("TRN2", target_bir_lowering=False, debug=False, num_devices=8)

    # x chunk-major: [sblock, ktile, 128, 512]
    xt_d = nc.dram_tensor("xt", [QB, KT, 128, SB], BF16, kind="ExternalInput").ap()
    # per-ktile weight wall: q(0:192) | k(192:384) | v(384:576) | qk_tail(576:704)
    wall_d = nc.dram_tensor("wall", [KT, 128, WQK], BF16, kind="ExternalInput").ap()
    wo_d = nc.dram_tensor("wo", [DKL, D], BF16, kind="ExternalInput").ap()
    ones_d = nc.dram_tensor("ones3", [128, NH, 1], BF16, kind="ExternalInput").ap()
    out_d = nc.dram_tensor("outT", [D, S], BF16, kind="ExternalOutput").ap()

    EXP = mybir.ActivationFunctionType.Exp
    GE = mybir.AluOpType.is_ge
    MUL = mybir.AluOpType.mult

    with tile.TileContext(nc) as tc:
        with (
            tc.tile_pool(name="xp", bufs=1) as xp,
            tc.tile_pool(name="wp", bufs=1) as wp,
            tc.tile_pool(name="qk", bufs=1) as qk,
            tc.tile_pool(name="vp", bufs=1) as vp,
            tc.tile_pool(name="ep", bufs=10) as ep,
            tc.tile_pool(name="ys", bufs=1) as ys,
            tc.tile_pool(name="dn", bufs=4) as dn,
            tc.tile_pool(name="op", bufs=4) as op,
            tc.tile_pool(name="psA", bufs=3, space="PSUM") as psA,
            tc.tile_pool(name="psB", bufs=3, space="PSUM") as psB,
            tc.tile_pool(name="psC", bufs=2, space="PSUM") as psC,
        ):
            nc.gpsimd.load_library(library_config.attn)

            # ---- Phase 0: input DMAs on two queues, first-needed first ----
            walls = []
            for k in range(KT):
                t = wp.tile([128, WQK], BF16, tag=f"wall{k}")
                walls.append(t)
            xts = []
            for k in range(KT):
                t = xp.tile([128, S], BF16, tag=f"x{k}")
                xts.append(t)

            # sync queue: weight walls, then x sblocks 2,3
            # gpsimd queue: x sblocks 0,1 (gate the prefix)
            for k in range(KT):
                nc.sync.dma_start(walls[k][:], wall_d[k])
            for n in (0, 1):
                for k in range(KT):
                    nc.gpsimd.dma_start(
                        xts[k][:, n * SB : (n + 1) * SB], xt_d[n, k]
                    )
            for n in (2, 3):
                for k in range(KT):
                    nc.sync.dma_start(
                        xts[k][:, n * SB : (n + 1) * SB], xt_d[n, k]
                    )
            # vector queue: c_proj weights (needed only mid-stream)
            wo1 = wp.tile([128, D], BF16, tag="wo1")
            nc.scalar.dma_start(wo1[:], wo_d[0:128])
            wo2 = wp.tile([64, D], BF16, tag="wo2")
            nc.scalar.dma_start(wo2[:], wo_d[128:DKL])

            # HAM warmup: keep the PE clock-gate open during the input-DMA
            # wait so the first real matmuls run at 2.4 GHz, not 1.2.
            warm = wp.tile([1, 640], BF16, tag="warm")
            nc.vector.memset(warm[:], 0.0)
            for i in range(40):
                wps = psC.tile([128, SB], F32, tag="mm", name=f"warm{i}")
                nc.tensor.matmul(
                    wps[:], warm[0:1, 0:128], warm[0:1, 128:640],
                    start=True, stop=True,
                )

            qA = qk.tile([128, S], BF16, tag="qA")
            qB_ = qk.tile([64, S], BF16, tag="qB")
            kA = qk.tile([128, S], BF16, tag="kA")
            kB = qk.tile([64, S], BF16, tag="kB")
            yA = ys.tile([128, S], BF16, tag="yA")
            yB = ys.tile([64, S], BF16, tag="yB")

            vas = [None] * TT

            def qkv_groups(ncol):
                c0, c1 = ncol * SB, (ncol + 1) * SB

                def g_q(n=ncol):
                    ps = psC.tile([128, SB], F32, tag="mm", name=f"pq_{n}")
                    for k in range(KT):
                        nc.tensor.matmul(
                            ps[:], walls[k][:, 0:128], xts[k][:, c0:c1],
                            start=(k == 0), stop=(k == KT - 1),
                        )
                    nc.vector.tensor_copy(qA[:, c0:c1], ps[:])

                def g_k(n=ncol):
                    ps = psC.tile([128, SB], F32, tag="mm", name=f"pk_{n}")
                    for k in range(KT):
                        nc.tensor.matmul(
                            ps[:], walls[k][:, 192:320], xts[k][:, c0:c1],
                            start=(k == 0), stop=(k == KT - 1),
                        )
                    nc.vector.tensor_copy(kA[:, c0:c1], ps[:])

                def g_tail(n=ncol):
                    # one full-array matmul: rows 0:64 = q cols 128:192,
                    # rows 64:128 = k cols 128:192 (concatenated weights)
                    ps = psC.tile([128, SB], F32, tag="mm", name=f"pt_{n}")
                    for k in range(KT):
                        nc.tensor.matmul(
                            ps[:], walls[k][:, 576:704], xts[k][:, c0:c1],
                            start=(k == 0), stop=(k == KT - 1),
                        )
                    nc.vector.tensor_copy(qB_[:, c0:c1], ps[0:64, :])
                    nc.vector.tensor_copy(kB[:, c0:c1], ps[64:128, :])

                def mk_v(t):
                    def g_v():
                        ps = psC.tile([128, SB], F32, tag="mm", name=f"pv{t}")
                        for k in range(KT):
                            nc.tensor.matmul(
                                ps[:, 0:DKL],
                                xts[k][:, t * 128 : (t + 1) * 128],
                                walls[k][:, 384:576],
                                start=(k == 0), stop=(k == KT - 1),
                            )
                        va = vp.tile(
                            [128, NH * (HD + 1)], BF16, tag=f"v{t}", name=f"va{t}"
                        )
                        var = va[:].rearrange("p (h c) -> p h c", c=HD + 1)
                        nc.gpsimd.dma_start(var[:, :, HD : HD + 1], ones_d)
                        nc.vector.tensor_copy(
                            var[:, :, 0:HD],
                            ps[:, 0:DKL].rearrange("p (h d) -> p h d", d=HD),
                        )
                        vas[t] = va
                    return g_v

                return [g_q, g_k, g_tail] + [mk_v(t) for t in range(4 * ncol, 4 * ncol + 4)]

            def cproj_groups(qb):
                def mk(mc):
                    def g():
                        ps = psC.tile([128, SB], F32, tag="mm", name=f"cp_{qb}_{mc}")
                        nc.tensor.matmul(
                            ps[:],
                            wo1[:, mc * 128 : (mc + 1) * 128],
                            yA[:, qb * SB : (qb + 1) * SB],
                            start=True, stop=False,
                        )
                        nc.tensor.matmul(
                            ps[:],
                            wo2[:, mc * 128 : (mc + 1) * 128],
                            yB[:, qb * SB : (qb + 1) * SB],
                            start=False, stop=True,
                        )
                        st = op.tile([128, SB], BF16, tag="st", name=f"st_{qb}_{mc}")
                        nc.vector.tensor_copy(st[:], ps[:])
                        nc.sync.dma_start(
                            out_d[mc * 128 : (mc + 1) * 128, qb * SB : (qb + 1) * SB],
                            st[:],
                        )
                    return g
                return [mk(mc) for mc in range(D // 128)]

            def att_tile(qb, t, yps, nt):
                d = t * 128 - qb * SB
                lo, sw = (d, 128) if d >= 0 else (0, 0)
                sq = qA[:, qb * SB + lo : (qb + 1) * SB]
                sqB = qB_[:, qb * SB + lo : (qb + 1) * SB]
                sps = [
                    psA.tile([128, SB], F32, tag="sc", name=f"sp_{qb}_{t}_{h}")
                    for h in range(NH)
                ]
                nc.tensor.matmul(
                    sps[0][:, lo:SB],
                    kA[0:64, t * 128 : (t + 1) * 128],
                    sq[0:64, :], start=True, stop=True,
                )
                nc.tensor.matmul(
                    sps[1][:, lo:SB],
                    kA[64:128, t * 128 : (t + 1) * 128],
                    sq[64:128, :], start=True, stop=True,
                )
                nc.tensor.matmul(
                    sps[2][:, lo:SB],
                    kB[0:64, t * 128 : (t + 1) * 128],
                    sqB[0:64, :], start=True, stop=True,
                )
                for h in range(NH):
                    ex = ep.tile([128, SB], BF16, tag="exp", name=f"ex_{qb}_{t}_{h}")
                    nc.scalar.activation(
                        ex[:, lo:SB], sps[h][:, lo:SB], EXP, scale=float(SCALE)
                    )
                    if d >= 0:
                        nc.gpsimd.affine_select(
                            out=ex[:, lo : lo + sw],
                            in_=ex[:, lo : lo + sw],
                            compare_op=GE, fill=0.0,
                            base=0, channel_multiplier=-1,
                            pattern=[[1, sw]],
                        )
                    nc.tensor.matmul(
                        yps[h][:, lo:SB],
                        vas[t][:, h * (HD + 1) : (h + 1) * (HD + 1)],
                        ex[:, lo:SB],
                        start=(t == 0), stop=(t == nt - 1),
                    )

            def divide_h(qb, h, yps):
                dr = dn.tile([1, SB], F32, tag="dr", name=f"dr{qb}{h}")
                nc.vector.tensor_copy(dr[:], yps[h][HD : HD + 1, :])
                rc = dn.tile([1, SB], F32, tag="rc", name=f"rc{qb}{h}")
                nc.vector.reciprocal_approx_fast(rc[:], dr[:])
                bc = dn.tile([64, SB], F32, tag="bc", name=f"bc{qb}{h}")
                nc.gpsimd.partition_broadcast(bc[:], rc[:], channels=64)
                if h == 0:
                    dst = yA[0:64, qb * SB : (qb + 1) * SB]
                elif h == 1:
                    dst = yA[64:128, qb * SB : (qb + 1) * SB]
                else:
                    dst = yB[0:64, qb * SB : (qb + 1) * SB]
                nc.vector.tensor_tensor(dst, yps[h][0:HD, :], bc[:], MUL)

            from collections import deque

            # prefix: projections for sblocks 0 and 1 (paced by input DMA)
            for g in qkv_groups(0):
                g()
            for g in qkv_groups(1):
                g()

            # attention windows: (qb, background PE work woven between tiles).
            # Order 1,3,2,0 so the last window is the small, PE-bound all-diag
            # qb=0 block and its c_proj tail runs on a warm PE.
            windows = [
                (0, list(qkv_groups(2))),
                (1, list(qkv_groups(3))),
                (3, cproj_groups(0) + cproj_groups(1)),
                (2, cproj_groups(3)),
            ]
            for qb, bgl in windows:
                bg = deque(bgl)
                nt = 4 * qb + 4
                yps = [
                    psB.tile([HD + 1, SB], F32, tag="ya", name=f"yps_{qb}_{h}")
                    for h in range(NH)
                ]
                nbg = len(bg)
                emitted = 0
                for i in range(nt):
                    att_tile(qb, i, yps, nt)
                    if i == nt - 1:
                        for h in range(NH):
                            divide_h(qb, h, yps)
                    want = (i + 1) * nbg // nt
                    while emitted < want and bg:
                        bg.popleft()()
                        emitted += 1
            for g in cproj_groups(2):
                g()

    nc.compile()
    return nc


def kernel(x, Wq, bq, Wk, bk, Wv, bv, Wo, bo):
    global LAST_EXEC_NS
    x = np.asarray(x, dtype=np.float32)
    Wq = np.asarray(Wq, dtype=np.float32)
    Wk = np.asarray(Wk, dtype=np.float32)
    Wv = np.asarray(Wv, dtype=np.float32)
    Wo = np.asarray(Wo, dtype=np.float32)
    bq = np.asarray(bq, dtype=np.float32)
    bk = np.asarray(bk, dtype=np.float32)
    bv = np.asarray(bv, dtype=np.float32)
    bo = np.asarray(bo, dtype=np.float32)

    # The device program folds no biases; handle the (unused in this problem)
    # nonzero case on the host by a reference fallback.
    if np.any(bq) or np.any(bk) or np.any(bv):
        q = (x @ Wq + bq).reshape(S, B, H, HD)
        k = (x @ Wk + bk).reshape(S, B, H, HD)
        v = (x @ Wv + bv).reshape(S, B, H, HD)
        att = np.einsum("sbhd,tbhd->bhst", q, k) * SCALE
        causal = np.triu(np.ones((S, S), dtype=bool), k=1)
        att = np.where(causal[None, None], -np.inf, att)
        att = att - att.max(axis=-1, keepdims=True)
        att = np.exp(att)
        att = att / att.sum(axis=-1, keepdims=True)
        y = np.einsum("bhst,tbhd->sbhd", att, v).reshape(S, B, D)
        return (y @ Wo + bo).astype(np.float32)

    if "prog" not in _PROGRAM_CACHE:
        _PROGRAM_CACHE["prog"] = _build_program()
    nc = _PROGRAM_CACHE["prog"]

    import ml_dtypes

    bf = ml_dtypes.bfloat16
    in_maps = []
    # x chunk-major: [sblock, ktile, 128, 512]
    xT = [
        np.ascontiguousarray(
            x[:, b, :].T.astype(bf).reshape(KT, 128, QB, SB).transpose(2, 0, 1, 3)
        )
        for b in range(B)
    ]
    for c in range(8):
        b, g = c // 4, c % 4
        sl = slice(g * DKL, (g + 1) * DKL)
        wq_l = Wq[:, sl].astype(bf).reshape(KT, 128, DKL)
        wk_l = Wk[:, sl].astype(bf).reshape(KT, 128, DKL)
        wv_l = Wv[:, sl].astype(bf).reshape(KT, 128, DKL)
        wall = np.ascontiguousarray(
            np.concatenate(
                [wq_l, wk_l, wv_l, wq_l[:, :, 128:], wk_l[:, :, 128:]], axis=2
            )
        )
        in_maps.append({
            "xt": xT[b],
            "wall": wall,
            "wo": np.ascontiguousarray(Wo[sl, :]).astype(bf),
            "ones3": np.ones((128, NH, 1), dtype=bf),
        })

    trace = bool(int(os.environ.get("KERNEL_TRACE", "0")))
    res = bass_utils.run_bass_kernel_spmd(
        nc, in_maps, core_ids=list(range(8)), trace=trace
    )
    LAST_EXEC_NS = res.exec_time_ns

    out = np.zeros((S, B, D), dtype=np.float32)
    for c in range(8):
        b = c // 4
        out[:, b, :] += res.results[c]["outT"].astype(np.float32).T
    out += bo
    return out


# revision 23
# speedup vs baseline: 1.2958x; 1.0893x over previous
"""Causal self-attention kernel for Trainium2, sharded over 8 NeuronCores.

Problem: x:(2048,2,768) f32, 12 heads, head_dim 64.
Sharding: batch (2) x head-groups (4 groups of 3 heads) -> 8 cores.
Each core computes q/k/v projections for its (batch, 3 heads), causal
flash-style attention, and a partial c_proj contribution. The host sums the
4 partial outputs per batch (the "all-reduce") and adds bo.

Device-side layout notes:
  - Matmul operands are bf16 (1 cycle/row on PE; fp32 accumulate in PSUM).
  - Scores are computed TRANSPOSED: scoresT[t, s] so softmax's denominator
    comes from a ones-column appended to V (m=65 matmul) and the exp runs
    along the free axis; no PE transposes of the probability matrix needed.
  - Causal masking: diagonal 128x512 score tiles restrict the live column
    range (lo) and a gpsimd affine_select zeroes the triangular remainder.
  - x is DMA'd in (sblock, ktile) chunks across two queues so the first
    projection matmuls start as soon as chunk (0, sb0) lands; weights for
    q/k/v plus a concatenated [q_tail|k_tail] block arrive as one tile per
    ktile ("wall").
  - Query blocks are processed in order 0,1,3,2 with deferred c_proj blocks
    woven between attention tiles so the PE stays fed in the exp-paced
    stretches, and per-head divides start as soon as that head's last AV
    matmul retires.
"""

import os
import sys

sys.path.insert(0, "/opt/trn_rl_repo")

import numpy as np

import concourse.bass as bass  # noqa: F401  (import keeps bass registered)
import concourse.tile as tile
from concourse import bacc, bass_utils, library_config, mybir

F32 = mybir.dt.float32
BF16 = mybir.dt.bfloat16

S = 2048          # sequence length
B = 2             # batch
D = 768           # d_model
H = 12            # total heads
HD = 64           # head dim
NH = 3            # heads per core
DKL = NH * HD     # local q/k/v width = 192
KT = 6            # k-tiles over D (6 x 128)
SB = 512          # s-block width
QB = S // SB      # 4 q-blocks
TT = S // 128     # 16 t-tiles
WQK = 3 * DKL + 128  # wall width: q|k|v|qtail|ktail = 704
SCALE = 1.0 / np.sqrt(HD)

_PROGRAM_CACHE = {}
LAST_EXEC_NS = None


def _build_program():
    nc = bacc.Bacc("TRN2", target_bir_lowering=False, debug=False, num_devices=8)

    # x chunk-major: [sblock, ktile, 128, 512]
    xt_d = nc.dram_tensor("xt", [QB, KT, 128, SB], BF16, kind="ExternalInput").ap()
    # per-ktile weight wall: q(0:192) | k(192:384) | v(384:576) | qk_tail(576:704)
    wall_d = nc.dram_tensor("wall", [KT, 128, WQK], BF16, kind="ExternalInput").ap()
    wo_d = nc.dram_tensor("wo", [DKL, D], BF16, kind="ExternalInput").ap()
    ones_d = nc.dram_tensor("ones3", [128, NH, 1], BF16, kind="ExternalInput").ap()
    out_d = nc.dram_tensor("outT", [D, S], BF16, kind="ExternalOutput").ap()

    EXP = mybir.ActivationFunctionType.Exp
    GE = mybir.AluOpType.is_ge
    MUL = mybir.AluOpType.mult

    with tile.TileContext(nc) as tc:
        with (
            tc.tile_pool(name="xp", bufs=1) as xp,
            tc.tile_pool(name="wp", bufs=1) as wp,
            tc.tile_pool(name="qk", bufs=1) as qk,
            tc.tile_pool(name="vp", bufs=1) as vp,
            tc.tile_pool(name="ep", bufs=10) as ep,
            tc.tile_pool(name="ys", bufs=1) as ys,
            tc.tile_pool(name="dn", bufs=4) as dn,
            tc.tile_pool(name="op", bufs=4) as op,
            tc.tile_pool(name="psA", bufs=3, space="PSUM") as psA,
            tc.tile_pool(name="psB", bufs=3, space="PSUM") as psB,
            tc.tile_pool(name="psC", bufs=2, space="PSUM") as psC,
        ):
            nc.gpsimd.load_library(library_config.attn)

            # ---- Phase 0: input DMAs on two queues, first-needed first ----
            walls = []
            for k in range(KT):
                t = wp.tile([128, WQK], BF16, tag=f"wall{k}")
                walls.append(t)
            xts = []
            for k in range(KT):
                t = xp.tile([128, S], BF16, tag=f"x{k}")
                xts.append(t)

            # hardware-DGE queues only (gpsimd SWDGE DMAs raced on HW):
            # scalar queue: x sblocks 0,1 (gate the prefix); sync: walls + x 2,3
            for k in range(KT):
                nc.sync.dma_start(walls[k][:], wall_d[k])
            for n in (0, 1):
                for k in range(KT):
                    nc.scalar.dma_start(
                        xts[k][:, n * SB : (n + 1) * SB], xt_d[n, k]
                    )
            for n in (2, 3):
                for k in range(KT):
                    nc.sync.dma_start(
                        xts[k][:, n * SB : (n + 1) * SB], xt_d[n, k]
                    )
            # vector queue: c_proj weights (needed only mid-stream)
            wo1 = wp.tile([128, D], BF16, tag="wo1")
            nc.scalar.dma_start(wo1[:], wo_d[0:128])
            wo2 = wp.tile([64, D], BF16, tag="wo2")
            nc.scalar.dma_start(wo2[:], wo_d[128:DKL])

            qA = qk.tile([128, S], BF16, tag="qA")
            qB_ = qk.tile([64, S], BF16, tag="qB")
            kA = qk.tile([128, S], BF16, tag="kA")
            kB = qk.tile([64, S], BF16, tag="kB")
            yA = ys.tile([128, S], BF16, tag="yA")
            yB = ys.tile([64, S], BF16, tag="yB")

            vas = [None] * TT

            def qkv_groups(ncol):
                c0, c1 = ncol * SB, (ncol + 1) * SB

                def g_q(n=ncol):
                    ps = psC.tile([128, SB], F32, tag="mm", name=f"pq_{n}")
                    for k in range(KT):
                        nc.tensor.matmul(
                            ps[:], walls[k][:, 0:128], xts[k][:, c0:c1],
                            start=(k == 0), stop=(k == KT - 1),
                        )
                    nc.vector.tensor_copy(qA[:, c0:c1], ps[:])

                def g_k(n=ncol):
                    ps = psC.tile([128, SB], F32, tag="mm", name=f"pk_{n}")
                    for k in range(KT):
                        nc.tensor.matmul(
                            ps[:], walls[k][:, 192:320], xts[k][:, c0:c1],
                            start=(k == 0), stop=(k == KT - 1),
                        )
                    nc.vector.tensor_copy(kA[:, c0:c1], ps[:])

                def g_tail(n=ncol):
                    # one full-array matmul: rows 0:64 = q cols 128:192,
                    # rows 64:128 = k cols 128:192 (concatenated weights)
                    ps = psC.tile([128, SB], F32, tag="mm", name=f"pt_{n}")
                    for k in range(KT):
                        nc.tensor.matmul(
                            ps[:], walls[k][:, 576:704], xts[k][:, c0:c1],
                            start=(k == 0), stop=(k == KT - 1),
                        )
                    nc.vector.tensor_copy(qB_[:, c0:c1], ps[0:64, :])
                    nc.vector.tensor_copy(kB[:, c0:c1], ps[64:128, :])

                def mk_v(t):
                    def g_v():
                        ps = psC.tile([128, SB], F32, tag="mm", name=f"pv{t}")
                        for k in range(KT):
                            nc.tensor.matmul(
                                ps[:, 0:DKL],
                                xts[k][:, t * 128 : (t + 1) * 128],
                                walls[k][:, 384:576],
                                start=(k == 0), stop=(k == KT - 1),
                            )
                        va = vp.tile(
                            [128, NH * (HD + 1)], BF16, tag=f"v{t}", name=f"va{t}"
                        )
                        var = va[:].rearrange("p (h c) -> p h c", c=HD + 1)
                        nc.sync.dma_start(var[:, :, HD : HD + 1], ones_d)
                        nc.vector.tensor_copy(
                            var[:, :, 0:HD],
                            ps[:, 0:DKL].rearrange("p (h d) -> p h d", d=HD),
                        )
                        vas[t] = va
                    return g_v

                return [g_q, g_k, g_tail] + [mk_v(t) for t in range(4 * ncol, 4 * ncol + 4)]

            def cproj_groups(qb):
                def mk(mc):
                    def g():
                        ps = psC.tile([128, SB], F32, tag="mm", name=f"cp_{qb}_{mc}")
                        nc.tensor.matmul(
                            ps[:],
                            wo1[:, mc * 128 : (mc + 1) * 128],
                            yA[:, qb * SB : (qb + 1) * SB],
                            start=True, stop=False,
                        )
                        nc.tensor.matmul(
                            ps[:],
                            wo2[:, mc * 128 : (mc + 1) * 128],
                            yB[:, qb * SB : (qb + 1) * SB],
                            start=False, stop=True,
                        )
                        st = op.tile([128, SB], BF16, tag="st", name=f"st_{qb}_{mc}")
                        nc.vector.tensor_copy(st[:], ps[:])
                        nc.sync.dma_start(
                            out_d[mc * 128 : (mc + 1) * 128, qb * SB : (qb + 1) * SB],
                            st[:],
                        )
                    return g
                return [mk(mc) for mc in range(D // 128)]

            def att_tile(qb, t, yps, nt):
                d = t * 128 - qb * SB
                lo, sw = (d, 128) if d >= 0 else (0, 0)
                sq = qA[:, qb * SB + lo : (qb + 1) * SB]
                sqB = qB_[:, qb * SB + lo : (qb + 1) * SB]
                sps = [
                    psA.tile([128, SB], F32, tag="sc", name=f"sp_{qb}_{t}_{h}")
                    for h in range(NH)
                ]
                nc.tensor.matmul(
                    sps[0][:, lo:SB],
                    kA[0:64, t * 128 : (t + 1) * 128],
                    sq[0:64, :], start=True, stop=True,
                )
                nc.tensor.matmul(
                    sps[1][:, lo:SB],
                    kA[64:128, t * 128 : (t + 1) * 128],
                    sq[64:128, :], start=True, stop=True,
                )
                nc.tensor.matmul(
                    sps[2][:, lo:SB],
                    kB[0:64, t * 128 : (t + 1) * 128],
                    sqB[0:64, :], start=True, stop=True,
                )
                for h in range(NH):
                    ex = ep.tile([128, SB], BF16, tag="exp", name=f"ex_{qb}_{t}_{h}")
                    nc.scalar.activation(
                        ex[:, lo:SB], sps[h][:, lo:SB], EXP, scale=float(SCALE)
                    )
                    if d >= 0:
                        nc.gpsimd.affine_select(
                            out=ex[:, lo : lo + sw],
                            in_=ex[:, lo : lo + sw],
                            compare_op=GE, fill=0.0,
                            base=0, channel_multiplier=-1,
                            pattern=[[1, sw]],
                        )
                    nc.tensor.matmul(
                        yps[h][:, lo:SB],
                        vas[t][:, h * (HD + 1) : (h + 1) * (HD + 1)],
                        ex[:, lo:SB],
                        start=(t == 0), stop=(t == nt - 1),
                    )

            def divide_h(qb, h, yps):
                dr = dn.tile([1, SB], F32, tag="dr", name=f"dr{qb}{h}")
                nc.vector.tensor_copy(dr[:], yps[h][HD : HD + 1, :])
                rc = dn.tile([1, SB], F32, tag="rc", name=f"rc{qb}{h}")
                nc.vector.reciprocal_approx_fast(rc[:], dr[:])
                bc = dn.tile([64, SB], F32, tag="bc", name=f"bc{qb}{h}")
                nc.gpsimd.partition_broadcast(bc[:], rc[:], channels=64)
                if h == 0:
                    dst = yA[0:64, qb * SB : (qb + 1) * SB]
                elif h == 1:
                    dst = yA[64:128, qb * SB : (qb + 1) * SB]
                else:
                    dst = yB[0:64, qb * SB : (qb + 1) * SB]
                nc.vector.tensor_tensor(dst, yps[h][0:HD, :], bc[:], MUL)

            from collections import deque

            # prefix: projections for sblocks 0 and 1 (paced by input DMA)
            for g in qkv_groups(0):
                g()
            for g in qkv_groups(1):
                g()

            # attention windows: (qb, background PE work woven between tiles).
            # Order 1,3,2,0 so the last window is the small, PE-bound all-diag
            # qb=0 block and its c_proj tail runs on a warm PE.
            windows = [
                (0, list(qkv_groups(2))),
                (1, list(qkv_groups(3))),
                (3, cproj_groups(0) + cproj_groups(1)),
                (2, cproj_groups(3)),
            ]
            for qb, bgl in windows:
                bg = deque(bgl)
                nt = 4 * qb + 4
                yps = [
                    psB.tile([HD + 1, SB], F32, tag="ya", name=f"yps_{qb}_{h}")
                    for h in range(NH)
                ]
                nbg = len(bg)
                emitted = 0
                for i in range(nt):
                    att_tile(qb, i, yps, nt)
                    if i == nt - 1:
                        for h in range(NH):
                            divide_h(qb, h, yps)
                    want = (i + 1) * nbg // nt
                    while emitted < want and bg:
                        bg.popleft()()
                        emitted += 1
            for g in cproj_groups(2):
                g()

    nc.compile()
    return nc


def kernel(x, Wq, bq, Wk, bk, Wv, bv, Wo, bo):
    global LAST_EXEC_NS
    x = np.asarray(x, dtype=np.float32)
    Wq = np.asarray(Wq, dtype=np.float32)
    Wk = np.asarray(Wk, dtype=np.float32)
    Wv = np.asarray(Wv, dtype=np.float32)
    Wo = np.asarray(Wo, dtype=np.float32)
    bq = np.asarray(bq, dtype=np.float32)
    bk = np.asarray(bk, dtype=np.float32)
    bv = np.asarray(bv, dtype=np.float32)
    bo = np.asarray(bo, dtype=np.float32)

    # The device program folds no biases; handle the (unused in this problem)
    # nonzero case on the host by a reference fallback.
    if np.any(bq) or np.any(bk) or np.any(bv):
        q = (x @ Wq + bq).reshape(S, B, H, HD)
        k = (x @ Wk + bk).reshape(S, B, H, HD)
        v = (x @ Wv + bv).reshape(S, B, H, HD)
        att = np.einsum("sbhd,tbhd->bhst", q, k) * SCALE
        causal = np.triu(np.ones((S, S), dtype=bool), k=1)
        att = np.where(causal[None, None], -np.inf, att)
        att = att - att.max(axis=-1, keepdims=True)
        att = np.exp(att)
        att = att / att.sum(axis=-1, keepdims=True)
        y = np.einsum("bhst,tbhd->sbhd", att, v).reshape(S, B, D)
        return (y @ Wo + bo).astype(np.float32)

    if "prog" not in _PROGRAM_CACHE:
        _PROGRAM_CACHE["prog"] = _build_program()
    nc = _PROGRAM_CACHE["prog"]

    import ml_dtypes

    bf = ml_dtypes.bfloat16
    in_maps = []
    # x chunk-major: [sblock, ktile, 128, 512]
    xT = [
        np.ascontiguousarray(
            x[:, b, :].T.astype(bf).reshape(KT, 128, QB, SB).transpose(2, 0, 1, 3)
        )
        for b in range(B)
    ]
    for c in range(8):
        b, g = c // 4, c % 4
        sl = slice(g * DKL, (g + 1) * DKL)
        wq_l = Wq[:, sl].astype(bf).reshape(KT, 128, DKL)
        wk_l = Wk[:, sl].astype(bf).reshape(KT, 128, DKL)
        wv_l = Wv[:, sl].astype(bf).reshape(KT, 128, DKL)
        wall = np.ascontiguousarray(
            np.concatenate(
                [wq_l, wk_l, wv_l, wq_l[:, :, 128:], wk_l[:, :, 128:]], axis=2
            )
        )
        in_maps.append({
            "xt": xT[b],
            "wall": wall,
            "wo": np.ascontiguousarray(Wo[sl, :]).astype(bf),
            "ones3": np.ones((128, NH, 1), dtype=bf),
        })

    trace = bool(int(os.environ.get("KERNEL_TRACE", "0")))
    res = bass_utils.run_bass_kernel_spmd(
        nc, in_maps, core_ids=list(range(8)), trace=trace
    )
    LAST_EXEC_NS = res.exec_time_ns

    out = np.zeros((S, B, D), dtype=np.float32)
    for c in range(8):
        b = c // 4
        out[:, b, :] += res.results[c]["outT"].astype(np.float32).T
    out += bo
    return out
